# revision 16
# baseline (speedup 1.0000x reference)
import numpy as np

import concourse.bass as bass
import concourse.bacc as bacc
import concourse.tile as tile
from concourse import mybir
from concourse import bass_utils
from concourse.masks import make_identity
from concourse._compat import with_exitstack

F32 = mybir.dt.float32
F16 = mybir.dt.float16

D = 768          # model dim
DH = 3072        # mlp hidden
S = 2048         # tokens per core (batch entry)
B = 8            # batch == n cores
CHUNK = 512
NCHUNK = S // CHUNK   # 4
NTT = CHUNK // 128    # 4 token tiles per chunk
KD = D // 128         # 6
KH = DH // 128        # 24
EPS = 1e-5
LAM = 1.0507009873554804934193349852946
ALPHA = 1.6732632423543772848170429916717


@with_exitstack
def _body(ctx, tc):
    nc = tc.nc
    xd = nc.dram_tensor("x", (S, D), F32, kind="ExternalInput")
    w1d = nc.dram_tensor("w1t", (D, D), F16, kind="ExternalInput")
    w2d = nc.dram_tensor("w2t", (D, DH), F16, kind="ExternalInput")
    w3d = nc.dram_tensor("w3t", (DH, D), F16, kind="ExternalInput")
    btd = nc.dram_tensor("btl", (128, KD), F32, kind="ExternalInput")
    b1d = nc.dram_tensor("b1l", (128, KH), F32, kind="ExternalInput")
    b1md = nc.dram_tensor("b1lam", (128, KH), F32, kind="ExternalInput")
    cbd = nc.dram_tensor("cbl", (128, KD), F32, kind="ExternalInput")
    g2d = nc.dram_tensor("g2l", (128, KD), F32, kind="ExternalInput")
    outd = nc.dram_tensor("out", (S, D), F32, kind="ExternalOutput")

    consts = ctx.enter_context(tc.tile_pool(name="consts", bufs=1))

    # x tiles prefetch on the default ring; weights go on the gpsimd ring so
    # the first LN1 tile doesn't sit behind 10.6MB of weights.
    px = ctx.enter_context(tc.tile_pool(name="px", bufs=5))
    x_tiles = []
    for c in range(NCHUNK):
        for tt in range(NTT):
            r0 = (c * NTT + tt) * 128
            xt = px.tile([128, D], F32, name="x")
            nc.default_dma_engine.dma_start(out=xt, in_=xd[r0:r0 + 128, :])
            x_tiles.append(xt)

    w1s = []
    for kc in range(KD):
        w = consts.tile([128, D], F16, name=f"w1s{kc}")
        nc.gpsimd.dma_start(out=w, in_=w1d[kc * 128:(kc + 1) * 128, :])
        w1s.append(w)
    w2s = []
    for kc in range(KD):
        w = consts.tile([128, DH], F16, name=f"w2s{kc}")
        nc.gpsimd.dma_start(out=w, in_=w2d[kc * 128:(kc + 1) * 128, :])
        w2s.append(w)
    w3s = []
    for kc in range(KH):
        w = consts.tile([128, D], F16, name=f"w3s{kc}")
        nc.gpsimd.dma_start(out=w, in_=w3d[kc * 128:(kc + 1) * 128, :])
        w3s.append(w)

    btl = consts.tile([128, KD], F32)
    nc.gpsimd.dma_start(out=btl, in_=btd[:, :])
    b1l = consts.tile([128, KH], F32)
    nc.gpsimd.dma_start(out=b1l, in_=b1d[:, :])
    b1m = consts.tile([128, KH], F32)
    nc.gpsimd.dma_start(out=b1m, in_=b1md[:, :])
    cbl = consts.tile([128, KD], F32)
    nc.gpsimd.dma_start(out=cbl, in_=cbd[:, :])
    g2l = consts.tile([128, KD], F32)
    nc.gpsimd.dma_start(out=g2l, in_=g2d[:, :])

    id16 = consts.tile([128, 128], F16)
    make_identity(nc, id16)
    id32 = consts.tile([128, 128], F32)
    make_identity(nc, id32)
    ones_k = consts.tile([128, 1], F16)
    nc.vector.memset(ones_k, 1.0)
    ones_m = consts.tile([1, 128], F32)
    nc.vector.memset(ones_m, 1.0)
    eps128 = consts.tile([128, 1], F32)
    nc.vector.memset(eps128, EPS)
    zero128 = consts.tile([128, 1], F32)
    nc.vector.memset(zero128, 0.0)
    eps1 = consts.tile([1, 1], F32)
    nc.vector.memset(eps1, EPS)
    zero1 = consts.tile([1, 1], F32)
    nc.vector.memset(zero1, 0.0)

    pn = ctx.enter_context(tc.tile_pool(name="pn", bufs=5))
    pstat = ctx.enter_context(tc.tile_pool(name="pstat", bufs=8))
    pnT = ctx.enter_context(tc.tile_pool(name="pnT", bufs=6))
    pr = ctx.enter_context(tc.tile_pool(name="pr", bufs=6))
    prsq = ctx.enter_context(tc.tile_pool(name="prsq", bufs=6))
    pm = ctx.enter_context(tc.tile_pool(name="pm", bufs=8))
    ptiny = ctx.enter_context(tc.tile_pool(name="ptiny", bufs=1))
    psel = ctx.enter_context(tc.tile_pool(name="psel", bufs=4))
    ph = ctx.enter_context(tc.tile_pool(name="ph", bufs=24))
    pf0 = ctx.enter_context(tc.tile_pool(name="pf0", bufs=2))
    pfin = ctx.enter_context(tc.tile_pool(name="pfin", bufs=6))
    pout = ctx.enter_context(tc.tile_pool(name="pout", bufs=2))

    psmm = ctx.enter_context(tc.tile_pool(name="psmm", bufs=4, space="PSUM"))
    psbc = ctx.enter_context(tc.tile_pool(name="psbc", bufs=1, space="PSUM"))
    pssm = ctx.enter_context(tc.tile_pool(name="pssm", bufs=1, space="PSUM"))

    AF = mybir.ActivationFunctionType
    OP = mybir.AluOpType

    for c in range(NCHUNK):
        # ---- LN1 (token-major): n = (x - mu) * rsqrt(var + eps), cast f16
        n_tiles = []
        for tt in range(NTT):
            xt = x_tiles[c * NTT + tt]
            xr = xt.rearrange("p (s f) -> p s f", f=256)
            stats = pstat.tile([128, 3, 6], F32, name="st")
            for sg in range(3):
                nc.vector.bn_stats(out=stats[:, sg, :], in_=xr[:, sg, :])
            mv = pstat.tile([128, 2], F32, name="mv")
            nc.vector.bn_aggr(out=mv, in_=stats)
            lnv = pstat.tile([128, 1], F32, name="lnv")
            nc.scalar.activation(out=lnv, in_=mv[:, 1:2], func=AF.Ln,
                                 bias=eps128)
            rstd = pstat.tile([128, 1], F32, name="rstd")
            nc.scalar.activation(out=rstd, in_=lnv, func=AF.Exp, scale=-0.5,
                                 bias=zero128)
            nt = pn.tile([128, D], F16, name="n")
            nc.vector.tensor_scalar(
                out=nt, in0=xt, scalar1=mv[:, 0:1], scalar2=rstd,
                op0=OP.subtract, op1=OP.mult)
            n_tiles.append(nt)

        # ---- transpose n -> nT (feature-major), f16
        nT = []
        for dc in range(KD):
            ps = psmm.tile([128, CHUNK], F16, name="mm")
            for tt in range(NTT):
                nc.tensor.transpose(
                    ps[:, tt * 128:(tt + 1) * 128],
                    n_tiles[tt][:, dc * 128:(dc + 1) * 128], id16)
            t = pnT.tile([128, CHUNK], F16, name="nT")
            nc.scalar.activation(out=t, in_=ps, func=AF.Copy)
            nT.append(t)

        # ---- GEMM1: rT[o,t] = W~ @ nT + b~  (residual + LN1 affine folded)
        rT = []
        rsqT = []
        for oc in range(KD):
            ps = psmm.tile([128, CHUNK], F32, name="mm")
            for kc in range(KD):
                nc.tensor.matmul(
                    ps, w1s[kc][:, oc * 128:(oc + 1) * 128], nT[kc],
                    start=(kc == 0), stop=(kc == KD - 1))
            rt = pr.tile([128, CHUNK], F16, name="rT")
            nc.scalar.activation(out=rt, in_=ps, func=AF.Identity,
                                 bias=btl[:, oc:oc + 1])
            rT.append(rt)
            rq = prsq.tile([128, CHUNK], F16, name="rsq")
            nc.vector.tensor_mul(out=rq, in0=rt, in1=rt)
            rsqT.append(rq)

        # ---- LN2 stats via ones-matmul partition reduction
        sum_r = pssm.tile([1, CHUNK], F32, name="sumr")
        for oc in range(KD):
            nc.tensor.matmul(sum_r, ones_k, rT[oc],
                             start=(oc == 0), stop=(oc == KD - 1))
        sum_q = pssm.tile([1, CHUNK], F32, name="sumq")
        for oc in range(KD):
            nc.tensor.matmul(sum_q, ones_k, rsqT[oc],
                             start=(oc == 0), stop=(oc == KD - 1))

        mean = ptiny.tile([1, CHUNK], F32, name="mean")
        nc.vector.tensor_scalar_mul(out=mean, in0=sum_r, scalar1=1.0 / D)
        msq = ptiny.tile([1, CHUNK], F32, name="msq")
        nc.vector.tensor_mul(out=msq, in0=mean, in1=mean)
        var = ptiny.tile([1, CHUNK], F32, name="var")
        nc.vector.scalar_tensor_tensor(
            out=var, in0=sum_q, scalar=1.0 / D, in1=msq,
            op0=OP.mult, op1=OP.subtract)
        lnv2 = ptiny.tile([1, CHUNK], F32, name="lnv2")
        nc.scalar.activation(out=lnv2, in_=var, func=AF.Ln, bias=eps1)
        s_t = ptiny.tile([1, CHUNK], F32, name="s")
        nc.scalar.activation(out=s_t, in_=lnv2, func=AF.Exp, scale=-0.5,
                             bias=zero1)
        ms_t = ptiny.tile([1, CHUNK], F32, name="ms")
        nc.vector.scalar_tensor_tensor(
            out=ms_t, in0=mean, scalar=-1.0, in1=s_t,
            op0=OP.mult, op1=OP.mult)

        # broadcast per-token scalars across partitions via K=1 matmul
        s_b = psbc.tile([128, CHUNK], F32, name="sb")
        nc.tensor.matmul(s_b, ones_m, s_t, start=True, stop=True)
        ms_b = psbc.tile([128, CHUNK], F32, name="msb")
        nc.tensor.matmul(ms_b, ones_m, ms_t, start=True, stop=True)

        # ---- LN2 normalize: m = r*s + ms   (feature-major f16)
        m_tiles = []
        for oc in range(KD):
            t0 = prsq.tile([128, CHUNK], F16, name="rsq")
            nc.vector.tensor_mul(out=t0, in0=rT[oc], in1=s_b)
            mt = pm.tile([128, CHUNK], F16, name="m")
            nc.vector.tensor_add(out=mt, in0=t0, in1=ms_b)
            m_tiles.append(mt)

        # ---- GEMM2 + SELU:  h' = lam*relu(u) + lam*alpha*exp(min(u,0))
        # (the -lam*alpha constant is folded into GEMM3's output bias)
        h_tiles = []
        for hc in range(KH):
            ps = psmm.tile([128, CHUNK], F32, name="mm")
            for kc in range(KD):
                nc.tensor.matmul(
                    ps, w2s[kc][:, hc * 128:(hc + 1) * 128], m_tiles[kc],
                    start=(kc == 0), stop=(kc == KD - 1))
            a = psel.tile([128, CHUNK], F16, name="a")
            nc.scalar.activation(out=a, in_=ps, func=AF.Relu, scale=LAM,
                                 bias=b1m[:, hc:hc + 1])
            tm = psel.tile([128, CHUNK], F16, name="tm")
            nc.vector.tensor_scalar(
                out=tm, in0=ps, scalar1=b1l[:, hc:hc + 1], scalar2=0.0,
                op0=OP.add, op1=OP.min)
            e = psel.tile([128, CHUNK], F16, name="e")
            nc.scalar.activation(out=e, in_=tm, func=AF.Exp, bias=zero128)
            ht = ph.tile([128, CHUNK], F16, name="h")
            nc.vector.scalar_tensor_tensor(
                out=ht, in0=e, scalar=LAM * ALPHA, in1=a,
                op0=OP.mult, op1=OP.add)
            h_tiles.append(ht)

        # ---- GEMM3 + residual: fin = h' @ W2^T + cb + m*g2
        fin = []
        for oc in range(KD):
            ps = psmm.tile([128, CHUNK], F32, name="mm")
            for kc in range(KH):
                nc.tensor.matmul(
                    ps, w3s[kc][:, oc * 128:(oc + 1) * 128], h_tiles[kc],
                    start=(kc == 0), stop=(kc == KH - 1))
            f0 = pf0.tile([128, CHUNK], F32, name="f0")
            nc.scalar.activation(out=f0, in_=ps, func=AF.Identity,
                                 bias=cbl[:, oc:oc + 1])
            ft = pfin.tile([128, CHUNK], F32, name="fin")
            nc.vector.scalar_tensor_tensor(
                out=ft, in0=m_tiles[oc], scalar=g2l[:, oc:oc + 1], in1=f0,
                op0=OP.mult, op1=OP.add)
            fin.append(ft)

        # ---- transpose back to token-major and store
        for tt in range(NTT):
            ps1 = psmm.tile([128, CHUNK], F32, name="mm")
            for j in range(4):
                nc.tensor.transpose(
                    ps1[:, j * 128:(j + 1) * 128],
                    fin[j][:, tt * 128:(tt + 1) * 128], id32)
            ps2 = psmm.tile([128, 256], F32, name="mm")
            for j in range(4, 6):
                nc.tensor.transpose(
                    ps2[:, (j - 4) * 128:(j - 3) * 128],
                    fin[j][:, tt * 128:(tt + 1) * 128], id32)
            ot = pout.tile([128, D], F32, name="o")
            nc.scalar.activation(out=ot[:, 0:512], in_=ps1, func=AF.Copy)
            nc.scalar.activation(out=ot[:, 512:768], in_=ps2, func=AF.Copy)
            r0 = (c * NTT + tt) * 128
            nc.gpsimd.dma_start(out=outd[r0:r0 + 128, :], in_=ot)


_NC_CACHE = None


def _build():
    global _NC_CACHE
    if _NC_CACHE is None:
        nc = bacc.Bacc("TRN2")
        with tile.TileContext(nc) as tc:
            _body(tc)
        nc.finalize()
        _NC_CACHE = nc
    return _NC_CACHE


def _fold_weights(inputs):
    in_weight = np.asarray(inputs["in_weight"], np.float32)
    in_bias = np.asarray(inputs["in_bias"], np.float32)
    out_w = np.asarray(inputs["out_w"], np.float32)
    out_b = np.asarray(inputs["out_b"], np.float32)
    mlp_w1 = np.asarray(inputs["mlp_w1"], np.float32)
    mlp_b1 = np.asarray(inputs["mlp_b1"], np.float32)
    mlp_w2 = np.asarray(inputs["mlp_w2"], np.float32)
    mlp_b2 = np.asarray(inputs["mlp_b2"], np.float32)
    ln1_g = np.asarray(inputs["ln1_g"], np.float32)
    ln1_b = np.asarray(inputs["ln1_b"], np.float32)
    ln2_g = np.asarray(inputs["ln2_g"], np.float32)
    ln2_b = np.asarray(inputs["ln2_b"], np.float32)

    # value-projection slice of the fused qkv weight (q/k/attn are dead code:
    # reference uses V directly as head output and discards the masks)
    W = in_weight.reshape(12, 64, 3, D)
    Wv = W[:, :, 2, :].reshape(D, D)
    bv = in_bias.reshape(12, 64, 3)[:, :, 2].reshape(D)

    Wc = out_w @ Wv                      # [o, d]
    cvec = out_w @ bv + out_b            # [o]

    # r = n @ W1~^T + b~ ; n is the pure LN1-normalized x
    W1t = (ln1_g[:, None] * (Wc.T + np.eye(D, dtype=np.float32)))  # [d, o]
    bt = Wc @ ln1_b + ln1_b + cvec

    # hpre = m @ W2~^T + b1~ ; m is the pure LN2-normalized r
    W2t = ln2_g[:, None] * mlp_w1.T      # [d, 3072]
    b1t = mlp_w1 @ ln2_b + mlp_b1

    # out = h' @ W2^T + cb + m*g2 ; h' = selu(hpre) + lam*alpha
    W3t = mlp_w2.T.copy()                # [3072, o]
    cb = mlp_b2 + ln2_b - LAM * ALPHA * mlp_w2.sum(axis=1)

    def lay(v, k):
        return np.ascontiguousarray(v.reshape(k, 128).T).astype(np.float32)

    return {
        "w1t": W1t.astype(np.float16),
        "w2t": W2t.astype(np.float16),
        "w3t": np.ascontiguousarray(W3t).astype(np.float16),
        "btl": lay(bt, KD),
        "b1l": lay(b1t, KH),
        "b1lam": lay(LAM * b1t, KH),
        "cbl": lay(cb, KD),
        "g2l": lay(ln2_g, KD),
    }


def run(inputs, trace=False):
    patches = np.asarray(inputs["patches"], np.float32)
    wmap = _fold_weights(inputs)
    nc = _build()
    in_maps = []
    for b in range(B):
        m = {"x": np.ascontiguousarray(patches[b])}
        m.update(wmap)
        in_maps.append(m)
    res = bass_utils.run_bass_kernel_spmd(
        nc, in_maps, core_ids=list(range(B)), trace=trace)
    out = np.stack([res.results[i]["out"] for i in range(B)], axis=0)
    return out.astype(np.float32), res


def kernel(**inputs):
    out, _ = run(inputs, trace=False)
    return out
